# revision 18
# baseline (speedup 1.0000x reference)
"""CRF loss kernel for Trainium2 (8 NeuronCores, Bass/Tile) — v2 banded scan.

Forward algorithm in the exp domain: p <- diag(exp(emit_t)) @ E @ p with
E = exp(transitions) shared across timesteps.  v2 packs EIGHT groups of
sub-sequences into the 128 partitions (group g occupies partitions
[16g, 16g+16)) with a block-diagonal E — one PE matmul [K=128, N=cols] and one
full-width DVE multiply per step.  Each core runs 512 sub-chunks of L=8 steps
(+B=8 burn-in from an arbitrary positive state; Perron-Frobenius contraction
makes the direction converge in <8 steps to f32 rounding).  Log-scale
accounting happens only at chunk boundaries via column sums (alpha/beta):

    log rho_col = ln(beta) - ln(alpha);   logZ = sum + ln(u . v_end)

The per-step exp(emit) slices are produced by PE transposes straight into
PSUM (partition layout g*16+i, one [128,64] slice per step) and consumed
there by the DVE multiply — no eviction copies.

Gold path score: one-hot tags via a single tensor_tensor is_equal with
broadcast APs; pair-count and emission sums accumulate on PE as
[C | D2] = O^T @ [O_prev | F];  gold = <C, trans> + trace(D2).

Host work: shard inputs, build the block-diagonal transitions pattern, sum 8
per-core scalars, add two boundary terms.
"""

import math

import numpy as np

import concourse.bacc as bacc
import concourse.bass as bass
import concourse.tile as tile
from concourse import mybir
from concourse.bass_utils import run_bass_kernel_spmd
from concourse.masks import make_identity

# ---- problem constants (hardcoded per contract) ----
T = 32768
K = 16
NC = 8
TC = T // NC            # 4096 timesteps per core
G = 8                   # partition groups
SPG = 64                # sub-chunks per group -> 512 columns/core
COLS = G * SPG
L = TC // COLS          # 8 real steps per column
B = 8                   # burn-in steps
STEPS = B + L           # 16
WWIN = STEPS            # window rows per column (16)
NCHUNK = 4              # preamble pipeline chunks (4 w's each)
RS_L2 = 42              # rescale factor 2^-42 applied once at tau=B
START = 14
STOP = 15
NST = 2                 # scan streams (split over s')
SH = SPG // NST         # 32 columns per stream
FDT = mybir.dt.float32
BDT = mybir.dt.bfloat16
FWIN = (COLS - 1) * L + WWIN   # 4104 feats rows per core

_CACHE: dict = {}


def _build_kernel():
    nc = bacc.Bacc("TRN2", target_bir_lowering=False, debug=False, num_devices=NC)

    featsw = nc.dram_tensor("featsw", [FWIN, K], FDT, kind="ExternalInput").ap()
    tagsw = nc.dram_tensor("tagsw", [TC + 1], FDT, kind="ExternalInput").ap()
    transTB = nc.dram_tensor("transTB", [128, 128], FDT, kind="ExternalInput").ap()
    consts = nc.dram_tensor("consts", [128, 188], FDT, kind="ExternalInput").ap()
    out = nc.dram_tensor("out", [1, 8], FDT, kind="ExternalOutput").ap()

    with tile.TileContext(nc) as tc:
        with (
            tc.tile_pool(name="singles", bufs=1) as singles,
            tc.tile_pool(name="qps", bufs=2, space="PSUM") as qps,
            tc.tile_pool(name="dbp", bufs=1, space="PSUM") as dbp,
            tc.tile_pool(name="gps", bufs=1, space="PSUM") as gps,
            tc.tile_pool(name="sps", bufs=2, space="PSUM") as sps,
        ):
            # ---------------- small loads + constants (host-packed) ----------
            # consts cols: 0:128 ident128 | 128:144 iota16f | 144:152 gself
            #   | 152:154 initmv | 154:155 ones | 155:156 sel8(rows0..7)
            #   | 156:188.. trid rows 0:16 cols 156..188? -> trid packed at
            #   [0:16, 136:168] of a second region; see host packing below.
            csb = singles.tile([128, 188], FDT)
            nc.scalar.dma_start(out=csb, in_=consts)
            transTB_sb = singles.tile([128, 128], FDT)
            nc.sync.dma_start(out=transTB_sb, in_=transTB)
            ident128 = csb[:, 0:128]
            iota16f = csb[:, 128:144]
            gself = csb[:, 144:152]
            initmv_sb = csb[:, 152:154]
            ones16 = csb[0:K, 154:155]
            ones8 = csb[0:G, 154:155]
            sel8 = csb[0:G, 155:156]
            trid_sb = csb[0:K, 156:156 + 2 * K]
            gsel = singles.tile([128, G], BDT)
            nc.vector.tensor_copy(gsel, gself)
            initmv_b = singles.tile([128, 2], BDT)
            nc.vector.tensor_copy(initmv_b, initmv_sb)
            # ETB = exp(transTB): block-diagonal E^T stack, bf16 for 1-pass MMs.
            # First ACT op -> exp table load overlaps the big feats DMAs.
            ETB = singles.tile([128, 128], BDT)
            nc.scalar.activation(ETB, transTB_sb, mybir.ActivationFunctionType.Exp)

            # gold-side loads on the scalar-engine DMA queue (parallel to sync)
            tsb = singles.tile([128, 33], FDT)
            nc.scalar.dma_start(
                out=tsb,
                in_=bass.AP(tensor=tagsw.tensor, offset=0,
                            ap=[[32, 128], [1, 33]]),
            )
            OpF = singles.tile([128, 32, 2 * K], BDT)
            OpFf = singles.tile([128, 32, K], FDT)
            nc.scalar.dma_start(
                out=OpFf,
                in_=bass.AP(tensor=featsw.tensor, offset=B * K,
                            ap=[[32 * K, 128], [K, 32], [1, K]]),
            )
            nc.vector.tensor_copy(OpF[:, :, K:2 * K], OpFf)

            # ---------------- gold (preamble: PE/DVE otherwise idle) ----------
            O = singles.tile([128, 32, K], BDT)
            nc.vector.tensor_tensor(
                O, tsb[:, 1:33].unsqueeze(2).broadcast_to([128, 32, K]),
                iota16f.unsqueeze(1).broadcast_to([128, 32, K]),
                mybir.AluOpType.is_equal)
            nc.vector.tensor_tensor(
                OpF[:, :, 0:K],
                tsb[:, 0:32].unsqueeze(2).broadcast_to([128, 32, K]),
                iota16f.unsqueeze(1).broadcast_to([128, 32, K]),
                mybir.AluOpType.is_equal)
            g_ps = gps.tile([K, 2 * K], FDT)
            for w in range(32):
                nc.tensor.matmul(g_ps, O[:, w, :], OpF[:, w, :],
                                 start=(w == 0), stop=(w == 31))
            gtmp = singles.tile([K, 2 * K], FDT)
            gacc = singles.tile([K, 1], FDT)
            nc.vector.tensor_tensor(gtmp, g_ps, trid_sb, mybir.AluOpType.mult)
            nc.vector.reduce_sum(gacc, gtmp, axis=mybir.AxisListType.X)
            gp_ps = sps.tile([1, 1], FDT, tag="sp")
            nc.tensor.matmul(gp_ps, ones16, gacc, start=True, stop=True)

            # ---------------- feats window: DMA -> exp -> transpose to PSUM ----
            # column c=(g*SPG+s') covers t in [base+c*L, +L); window rows
            # w in [0,16) map to featsw row c*L + w (base offset -B applied
            # on host via zero-padding).
            raww = singles.tile([SPG, G, WWIN, K], FDT)     # [64, 8, 16, 16]
            expw = singles.tile([SPG, WWIN, G, K], FDT)     # (g,i) contig per w
            dbt0 = dbp.tile([128, 8, SPG], FDT, tag="db0")
            dbt1 = dbp.tile([128, 8, SPG], FDT, tag="db1")
            dbt = [dbt0, dbt1]
            dbs = singles.tile([128, WWIN, SPG], FDT)
            CW = WWIN // NCHUNK                              # 4 w's per chunk
            nc.sync.dma_start(
                out=raww,
                in_=bass.AP(tensor=featsw.tensor, offset=0,
                            ap=[[L * K, SPG], [SPG * L * K, G],
                                [1, WWIN * K]]),
            )
            for c in range(NCHUNK):
                nc.scalar.activation(
                    expw[:, c * CW:(c + 1) * CW, :, :].transpose([0, 2, 1, 3]),
                    raww[:, :, c * CW:(c + 1) * CW, :],
                    mybir.ActivationFunctionType.Exp)
                for w in range(c * CW, (c + 1) * CW):
                    # [64, (g,i)=128] -> [128, 64] slice of PSUM D tile
                    nc.tensor.transpose(
                        dbt[w // 8][:, w % 8, :],
                        expw[:, w, :, :],
                        ident128[0:SPG, 0:SPG])
                nc.vector.tensor_copy(
                    dbs[:, c * CW:(c + 1) * CW, :],
                    dbt[(c * CW) // 8][:, (c * CW) % 8:(c * CW) % 8 + CW, :])

            # ---------------- scan ----------------
            Pb = singles.tile([128, SPG], BDT)
            nc.vector.memset(Pb, 1.0)
            asb = singles.tile([G, SPG], FDT)
            bsb = singles.tile([G, SPG], FDT)
            ln_a = singles.tile([G, SPG], FDT)
            ln_b = singles.tile([G, SPG], FDT)
            sa = singles.tile([G, 1], FDT)
            sb2 = singles.tile([G, 1], FDT)

            rs_const = float(2.0 ** (-RS_L2))
            for tau in range(STEPS):
                if tau == B:
                    nc.vector.tensor_scalar_mul(Pb, Pb, rs_const)
                    # core 0 only (mask/value inputs): column (g=0, s'=0)
                    nc.vector.tensor_tensor(Pb[:, 0:1], Pb[:, 0:1],
                                            initmv_b[:, 0:1],
                                            mybir.AluOpType.mult)
                    nc.vector.tensor_add(Pb[:, 0:1], Pb[:, 0:1],
                                         initmv_b[:, 1:2])
                    alpha_ps = sps.tile([G, SPG], FDT, tag="sp")
                    nc.tensor.matmul(alpha_ps, gsel, Pb, start=True, stop=True)
                    nc.vector.tensor_copy(asb, alpha_ps)
                    nc.scalar.activation(ln_a, asb,
                                         mybir.ActivationFunctionType.Ln,
                                         accum_out=sa)
                for h in range(NST):
                    Ph = Pb[:, h * SH:(h + 1) * SH]
                    q = qps.tile([128, SH], FDT, tag="q")
                    nc.tensor.matmul(q, ETB, Ph, start=True, stop=True)
                    dsl = dbs[:, tau, h * SH:(h + 1) * SH]
                    nc.vector.tensor_tensor(Ph, q, dsl, mybir.AluOpType.mult)

            beta_ps = sps.tile([G, SPG], FDT, tag="sp")
            nc.tensor.matmul(beta_ps, gsel, Pb, start=True, stop=True)
            nc.vector.tensor_copy(bsb, beta_ps)

            # ---------------- epilogue ----------------
            nc.scalar.activation(ln_b, bsb, mybir.ActivationFunctionType.Ln,
                                 accum_out=sb2)
            d8 = singles.tile([G, 1], FDT)
            nc.vector.tensor_sub(d8, sb2, sa)
            fp_ps = sps.tile([1, 1], FDT, tag="sp")
            nc.tensor.matmul(fp_ps, ones8, d8, start=True, stop=True)

            # u . v_end: u = ETB[:, 127] (block g=7, row STOP); beta_last via sel8
            ud_ps = sps.tile([1, 1], FDT, tag="sp")
            nc.tensor.matmul(ud_ps, ETB[:, 127:128], Pb[:, SPG - 1:SPG],
                             start=True, stop=True)
            bl_ps = sps.tile([1, 1], FDT, tag="sp")
            nc.tensor.matmul(bl_ps, sel8, bsb[:, SPG - 1:SPG],
                             start=True, stop=True)
            ubl = singles.tile([1, 2], FDT)
            nc.vector.tensor_copy(ubl[:, 0:1], ud_ps)
            nc.vector.tensor_copy(ubl[:, 1:2], bl_ps)
            lnubl = singles.tile([1, 2], FDT)
            nc.scalar.activation(lnubl, ubl, mybir.ActivationFunctionType.Ln)

            osb = singles.tile([1, 8], FDT)
            nc.vector.memset(osb, 0.0)
            nc.vector.tensor_copy(osb[:, 0:1], fp_ps)
            nc.vector.tensor_copy(osb[:, 1:3], lnubl)
            nc.vector.tensor_copy(osb[:, 3:4], gp_ps)
            nc.sync.dma_start(out=out, in_=osb)

    nc.compile()
    return nc


def _get_nc():
    if "nc" not in _CACHE:
        _CACHE["nc"] = _build_kernel()
    return _CACHE["nc"]


def _make_in_maps(feats, tags, transitions):
    feats = np.ascontiguousarray(feats, dtype=np.float32)
    tags_i = np.asarray(tags).astype(np.int64)
    trans = np.ascontiguousarray(transitions, dtype=np.float32)

    featsP = np.vstack([np.zeros((B, K), np.float32), feats])
    tagsX = np.concatenate([[START], tags_i]).astype(np.float32)
    # block-diagonal transposed-transitions pattern; exp of -1e4 -> 0 off-blocks
    TB = np.full((128, 128), -10000.0, np.float32)
    for g in range(G):
        TB[g * K:(g + 1) * K, g * K:(g + 1) * K] = trans.T

    base_consts = np.zeros((128, 188), np.float32)
    base_consts[:, 0:128] = np.eye(128, dtype=np.float32)
    base_consts[:, 128:144] = np.arange(K, dtype=np.float32)[None, :]
    base_consts[:, 144:152] = (np.arange(128)[:, None] // K ==
                               np.arange(G)[None, :]).astype(np.float32)
    base_consts[:, 152] = 1.0              # initmv mask (non-core-0 default)
    base_consts[:, 154] = 1.0              # ones
    base_consts[7, 155] = 1.0              # sel8
    base_consts[0:K, 156:172] = trans
    base_consts[0:K, 172:188] = np.eye(K, dtype=np.float32)

    in_maps = []
    for c in range(NC):
        base = c * TC
        cc = base_consts.copy()
        if c == 0:
            cc[0:K, 152] = 0.0
            cc[START, 153] = 1.0       # column (g=0, s'=0) -> e_START
        in_maps.append({
            "featsw": np.ascontiguousarray(featsP[base: base + FWIN]),
            "tagsw": np.ascontiguousarray(tagsX[base: base + TC + 1]),
            "transTB": TB,
            "consts": cc,
        })
    return in_maps, tags_i, trans


def _combine(outs, tags_i, trans):
    fwd = sum(float(o["out"][0, 0]) for o in outs)
    logZ = fwd + float(outs[-1]["out"][0, 1]) - float(outs[-1]["out"][0, 2])
    gold = sum(float(o["out"][0, 3]) for o in outs)
    gold += float(trans[STOP, tags_i[-1]])
    return np.float32((logZ - gold) / T)


def kernel(feats, tags, transitions):
    nc = _get_nc()
    in_maps, tags_i, trans = _make_in_maps(feats, tags, transitions)
    res = run_bass_kernel_spmd(nc, in_maps, core_ids=list(range(NC)))
    return _combine(res.results, tags_i, trans)


if __name__ == "__main__":
    d = np.load("/root/problem/inputs_only.npz")
    loss = kernel(d["feats"], d["tags"], d["transitions"])
    print("loss:", loss)


# revision 20
# speedup vs baseline: 1.0682x; 1.0682x over previous
"""CRF loss kernel for Trainium2 (8 NeuronCores, Bass/Tile) — v2 banded scan.

Forward algorithm in the exp domain: p <- diag(exp(emit_t)) @ E @ p with
E = exp(transitions) shared across timesteps.  v2 packs EIGHT groups of
sub-sequences into the 128 partitions (group g occupies partitions
[16g, 16g+16)) with a block-diagonal E — one PE matmul [K=128, N=cols] and one
full-width DVE multiply per step.  Each core runs 512 sub-chunks of L=8 steps
(+B=8 burn-in from an arbitrary positive state; Perron-Frobenius contraction
makes the direction converge in <8 steps to f32 rounding).  Log-scale
accounting happens only at chunk boundaries via column sums (alpha/beta):

    log rho_col = ln(beta) - ln(alpha);   logZ = sum + ln(u . v_end)

The per-step exp(emit) slices are produced by PE transposes straight into
PSUM (partition layout g*16+i, one [128,64] slice per step) and consumed
there by the DVE multiply — no eviction copies.

Gold path score: one-hot tags via a single tensor_tensor is_equal with
broadcast APs; pair-count and emission sums accumulate on PE as
[C | D2] = O^T @ [O_prev | F];  gold = <C, trans> + trace(D2).

Host work: shard inputs, build the block-diagonal transitions pattern, sum 8
per-core scalars, add two boundary terms.
"""

import math

import numpy as np

import concourse.bacc as bacc
import concourse.bass as bass
import concourse.tile as tile
from concourse import mybir
from concourse.bass_utils import run_bass_kernel_spmd

# ---- problem constants (hardcoded per contract) ----
T = 32768
K = 16
NC = 8
TC = T // NC            # 4096 timesteps per core
G = 8                   # partition groups
SPG = 64                # sub-chunks per group -> 512 columns/core
COLS = G * SPG
L = TC // COLS          # 8 real steps per column
B = 8                   # burn-in steps
STEPS = B + L           # 16
WWIN = STEPS            # window rows per column (16)
NCHUNK = 4              # preamble pipeline chunks (4 w's each)
RS_L2 = 42              # rescale factor 2^-42 applied once at tau=B
START = 14
STOP = 15
NST = 2                 # scan streams (split over s')
SH = SPG // NST         # 32 columns per stream
FDT = mybir.dt.float32
BDT = mybir.dt.bfloat16
FWIN = (COLS - 1) * L + WWIN   # 4104 feats rows per core

_CACHE: dict = {}


def _build_kernel():
    nc = bacc.Bacc("TRN2", target_bir_lowering=False, debug=False, num_devices=NC)

    featsw = nc.dram_tensor("featsw", [FWIN, K], FDT, kind="ExternalInput").ap()
    tagsw = nc.dram_tensor("tagsw", [TC + 1], FDT, kind="ExternalInput").ap()
    transTB = nc.dram_tensor("transTB", [128, 128], FDT, kind="ExternalInput").ap()
    consts = nc.dram_tensor("consts", [128, 188], FDT, kind="ExternalInput").ap()
    out = nc.dram_tensor("out", [1, 8], FDT, kind="ExternalOutput").ap()

    with tile.TileContext(nc) as tc:
        with (
            tc.tile_pool(name="singles", bufs=1) as singles,
            tc.tile_pool(name="qps", bufs=2, space="PSUM") as qps,
            tc.tile_pool(name="dbp", bufs=1, space="PSUM") as dbp,
            tc.tile_pool(name="gps", bufs=1, space="PSUM") as gps,
            tc.tile_pool(name="sps", bufs=2, space="PSUM") as sps,
        ):
            # ---------------- small loads + constants (host-packed) ----------
            # consts cols: 0:128 ident128 | 128:144 iota16f | 144:152 gself
            #   | 152:154 initmv | 154:155 ones | 155:156 sel8(rows0..7)
            #   | 156:188.. trid rows 0:16 cols 156..188? -> trid packed at
            #   [0:16, 136:168] of a second region; see host packing below.
            csb = singles.tile([128, 188], FDT)
            nc.scalar.dma_start(out=csb, in_=consts)
            transTB_sb = singles.tile([128, 128], FDT)
            nc.sync.dma_start(out=transTB_sb, in_=transTB)
            ident128 = csb[:, 0:128]
            iota16f = csb[:, 128:144]
            gself = csb[:, 144:152]
            initmv_sb = csb[:, 152:154]
            ones16 = csb[0:K, 154:155]
            ones8 = csb[0:G, 154:155]
            sel8 = csb[0:G, 155:156]
            trid_sb = csb[0:K, 156:156 + 2 * K]
            gsel = singles.tile([128, G], BDT)
            nc.vector.tensor_copy(gsel, gself)
            initmv_b = singles.tile([128, 2], BDT)
            nc.vector.tensor_copy(initmv_b, initmv_sb)
            # ETB = exp(transTB): block-diagonal E^T stack, bf16 for 1-pass MMs.
            # First ACT op -> exp table load overlaps the big feats DMAs.
            ETB = singles.tile([128, 128], BDT)
            nc.scalar.activation(ETB, transTB_sb, mybir.ActivationFunctionType.Exp)

            # gold-side loads on the scalar-engine DMA queue (parallel to sync)
            tsb = singles.tile([128, 33], FDT)
            nc.gpsimd.dma_start(
                out=tsb,
                in_=bass.AP(tensor=tagsw.tensor, offset=0,
                            ap=[[32, 128], [1, 33]]),
            )
            OpF = singles.tile([128, 32, 2 * K], BDT)
            OpFf = singles.tile([128, 32, K], FDT)
            nc.gpsimd.dma_start(
                out=OpFf,
                in_=bass.AP(tensor=featsw.tensor, offset=B * K,
                            ap=[[32 * K, 128], [K, 32], [1, K]]),
            )
            nc.vector.tensor_copy(OpF[:, :, K:2 * K], OpFf)

            # ---------------- gold (preamble: PE/DVE otherwise idle) ----------
            O = singles.tile([128, 32, K], BDT)
            nc.vector.tensor_tensor(
                O, tsb[:, 1:33].unsqueeze(2).broadcast_to([128, 32, K]),
                iota16f.unsqueeze(1).broadcast_to([128, 32, K]),
                mybir.AluOpType.is_equal)
            nc.vector.tensor_tensor(
                OpF[:, :, 0:K],
                tsb[:, 0:32].unsqueeze(2).broadcast_to([128, 32, K]),
                iota16f.unsqueeze(1).broadcast_to([128, 32, K]),
                mybir.AluOpType.is_equal)
            g_ps = gps.tile([K, 2 * K], FDT)
            for w in range(32):
                nc.tensor.matmul(g_ps, O[:, w, :], OpF[:, w, :],
                                 start=(w == 0), stop=(w == 31))
            gtmp = singles.tile([K, 2 * K], FDT)
            gacc = singles.tile([K, 1], FDT)
            nc.vector.tensor_tensor(gtmp, g_ps, trid_sb, mybir.AluOpType.mult)
            nc.vector.reduce_sum(gacc, gtmp, axis=mybir.AxisListType.X)
            gp_ps = sps.tile([1, 1], FDT, tag="sp")
            nc.tensor.matmul(gp_ps, ones16, gacc, start=True, stop=True)

            # ---------------- feats window: DMA -> exp -> transpose to PSUM ----
            # column c=(g*SPG+s') covers t in [base+c*L, +L); window rows
            # w in [0,16) map to featsw row c*L + w (base offset -B applied
            # on host via zero-padding).
            raww = singles.tile([SPG, G, WWIN, K], FDT)     # [64, 8, 16, 16]
            expw = singles.tile([SPG, WWIN, G, K], FDT)     # (g,i) contig per w
            dbt0 = dbp.tile([128, 8, SPG], FDT, tag="db0")
            dbt1 = dbp.tile([128, 8, SPG], FDT, tag="db1")
            dbt = [dbt0, dbt1]
            dbs = singles.tile([128, WWIN, SPG], FDT)
            CW = WWIN // NCHUNK                              # 4 w's per chunk
            GH = G // 2
            nc.sync.dma_start(
                out=raww[:, 0:GH, :, :],
                in_=bass.AP(tensor=featsw.tensor, offset=0,
                            ap=[[L * K, SPG], [SPG * L * K, GH],
                                [1, WWIN * K]]),
            )
            nc.scalar.dma_start(
                out=raww[:, GH:G, :, :],
                in_=bass.AP(tensor=featsw.tensor, offset=GH * SPG * L * K,
                            ap=[[L * K, SPG], [SPG * L * K, GH],
                                [1, WWIN * K]]),
            )
            for c in range(NCHUNK):
                nc.scalar.activation(
                    expw[:, c * CW:(c + 1) * CW, :, :].transpose([0, 2, 1, 3]),
                    raww[:, :, c * CW:(c + 1) * CW, :],
                    mybir.ActivationFunctionType.Exp)
                for w in range(c * CW, (c + 1) * CW):
                    # [64, (g,i)=128] -> [128, 64] slice of PSUM D tile
                    nc.tensor.transpose(
                        dbt[w // 8][:, w % 8, :],
                        expw[:, w, :, :],
                        ident128[0:SPG, 0:SPG])
                nc.vector.tensor_copy(
                    dbs[:, c * CW:(c + 1) * CW, :],
                    dbt[(c * CW) // 8][:, (c * CW) % 8:(c * CW) % 8 + CW, :])

            # ---------------- scan ----------------
            Pb = singles.tile([128, SPG], BDT)
            nc.vector.memset(Pb, 1.0)
            asb = singles.tile([G, SPG], FDT)
            bsb = singles.tile([G, SPG], FDT)
            ln_a = singles.tile([G, SPG], FDT)
            ln_b = singles.tile([G, SPG], FDT)
            sa = singles.tile([G, 1], FDT)
            sb2 = singles.tile([G, 1], FDT)

            rs_const = float(2.0 ** (-RS_L2))
            for tau in range(STEPS):
                if tau == B:
                    nc.vector.tensor_scalar_mul(Pb, Pb, rs_const)
                    # core 0 only (mask/value inputs): column (g=0, s'=0)
                    nc.vector.tensor_tensor(Pb[:, 0:1], Pb[:, 0:1],
                                            initmv_b[:, 0:1],
                                            mybir.AluOpType.mult)
                    nc.vector.tensor_add(Pb[:, 0:1], Pb[:, 0:1],
                                         initmv_b[:, 1:2])
                    alpha_ps = sps.tile([G, SPG], FDT, tag="sp")
                    nc.tensor.matmul(alpha_ps, gsel, Pb, start=True, stop=True)
                    nc.vector.tensor_copy(asb, alpha_ps)
                    nc.scalar.activation(ln_a, asb,
                                         mybir.ActivationFunctionType.Ln,
                                         accum_out=sa)
                for h in range(NST):
                    Ph = Pb[:, h * SH:(h + 1) * SH]
                    q = qps.tile([128, SH], FDT, tag="q")
                    nc.tensor.matmul(q, ETB, Ph, start=True, stop=True)
                    dsl = dbs[:, tau, h * SH:(h + 1) * SH]
                    nc.vector.tensor_tensor(Ph, q, dsl, mybir.AluOpType.mult)

            beta_ps = sps.tile([G, SPG], FDT, tag="sp")
            nc.tensor.matmul(beta_ps, gsel, Pb, start=True, stop=True)
            nc.vector.tensor_copy(bsb, beta_ps)

            # ---------------- epilogue ----------------
            nc.scalar.activation(ln_b, bsb, mybir.ActivationFunctionType.Ln,
                                 accum_out=sb2)
            d8 = singles.tile([G, 1], FDT)
            nc.vector.tensor_sub(d8, sb2, sa)
            fp_ps = sps.tile([1, 1], FDT, tag="sp")
            nc.tensor.matmul(fp_ps, ones8, d8, start=True, stop=True)

            # u . v_end: u = ETB[:, 127] (block g=7, row STOP); beta_last via sel8
            ud_ps = sps.tile([1, 1], FDT, tag="sp")
            nc.tensor.matmul(ud_ps, ETB[:, 127:128], Pb[:, SPG - 1:SPG],
                             start=True, stop=True)
            bl_ps = sps.tile([1, 1], FDT, tag="sp")
            nc.tensor.matmul(bl_ps, sel8, bsb[:, SPG - 1:SPG],
                             start=True, stop=True)
            ubl = singles.tile([1, 2], FDT)
            nc.vector.tensor_copy(ubl[:, 0:1], ud_ps)
            nc.vector.tensor_copy(ubl[:, 1:2], bl_ps)
            lnubl = singles.tile([1, 2], FDT)
            nc.scalar.activation(lnubl, ubl, mybir.ActivationFunctionType.Ln)

            osb = singles.tile([1, 8], FDT)
            nc.vector.memset(osb, 0.0)
            nc.vector.tensor_copy(osb[:, 0:1], fp_ps)
            nc.vector.tensor_copy(osb[:, 1:3], lnubl)
            nc.vector.tensor_copy(osb[:, 3:4], gp_ps)
            nc.sync.dma_start(out=out, in_=osb)

    nc.compile()
    return nc


def _get_nc():
    if "nc" not in _CACHE:
        _CACHE["nc"] = _build_kernel()
    return _CACHE["nc"]


def _make_in_maps(feats, tags, transitions):
    feats = np.ascontiguousarray(feats, dtype=np.float32)
    tags_i = np.asarray(tags).astype(np.int64)
    trans = np.ascontiguousarray(transitions, dtype=np.float32)

    featsP = np.vstack([np.zeros((B, K), np.float32), feats])
    tagsX = np.concatenate([[START], tags_i]).astype(np.float32)
    # block-diagonal transposed-transitions pattern; exp of -1e4 -> 0 off-blocks
    TB = np.full((128, 128), -10000.0, np.float32)
    for g in range(G):
        TB[g * K:(g + 1) * K, g * K:(g + 1) * K] = trans.T

    base_consts = np.zeros((128, 188), np.float32)
    base_consts[:, 0:128] = np.eye(128, dtype=np.float32)
    base_consts[:, 128:144] = np.arange(K, dtype=np.float32)[None, :]
    base_consts[:, 144:152] = (np.arange(128)[:, None] // K ==
                               np.arange(G)[None, :]).astype(np.float32)
    base_consts[:, 152] = 1.0              # initmv mask (non-core-0 default)
    base_consts[:, 154] = 1.0              # ones
    base_consts[7, 155] = 1.0              # sel8
    base_consts[0:K, 156:172] = trans
    base_consts[0:K, 172:188] = np.eye(K, dtype=np.float32)

    in_maps = []
    for c in range(NC):
        base = c * TC
        cc = base_consts.copy()
        if c == 0:
            cc[0:K, 152] = 0.0
            cc[START, 153] = 1.0       # column (g=0, s'=0) -> e_START
        in_maps.append({
            "featsw": np.ascontiguousarray(featsP[base: base + FWIN]),
            "tagsw": np.ascontiguousarray(tagsX[base: base + TC + 1]),
            "transTB": TB,
            "consts": cc,
        })
    return in_maps, tags_i, trans


def _combine(outs, tags_i, trans):
    fwd = sum(float(o["out"][0, 0]) for o in outs)
    logZ = fwd + float(outs[-1]["out"][0, 1]) - float(outs[-1]["out"][0, 2])
    gold = sum(float(o["out"][0, 3]) for o in outs)
    gold += float(trans[STOP, tags_i[-1]])
    return np.float32((logZ - gold) / T)


def kernel(feats, tags, transitions):
    nc = _get_nc()
    in_maps, tags_i, trans = _make_in_maps(feats, tags, transitions)
    res = run_bass_kernel_spmd(nc, in_maps, core_ids=list(range(NC)))
    return _combine(res.results, tags_i, trans)


if __name__ == "__main__":
    d = np.load("/root/problem/inputs_only.npz")
    loss = kernel(d["feats"], d["tags"], d["transitions"])
    print("loss:", loss)


# revision 22
# speedup vs baseline: 1.1432x; 1.0702x over previous
"""CRF loss kernel for Trainium2 (8 NeuronCores, Bass/Tile) — v2 banded scan.

Forward algorithm in the exp domain: p <- diag(exp(emit_t)) @ E @ p with
E = exp(transitions) shared across timesteps.  v2 packs EIGHT groups of
sub-sequences into the 128 partitions (group g occupies partitions
[16g, 16g+16)) with a block-diagonal E — one PE matmul [K=128, N=cols] and one
full-width DVE multiply per step.  Each core runs 512 sub-chunks of L=8 steps
(+B=8 burn-in from an arbitrary positive state; Perron-Frobenius contraction
makes the direction converge in <8 steps to f32 rounding).  Log-scale
accounting happens only at chunk boundaries via column sums (alpha/beta):

    log rho_col = ln(beta) - ln(alpha);   logZ = sum + ln(u . v_end)

The per-step exp(emit) slices are produced by PE transposes straight into
PSUM (partition layout g*16+i, one [128,64] slice per step) and consumed
there by the DVE multiply — no eviction copies.

Gold path score: one-hot tags via a single tensor_tensor is_equal with
broadcast APs; pair-count and emission sums accumulate on PE as
[C | D2] = O^T @ [O_prev | F];  gold = <C, trans> + trace(D2).

Host work: shard inputs, build the block-diagonal transitions pattern, sum 8
per-core scalars, add two boundary terms.
"""

import math

import numpy as np

import concourse.bacc as bacc
import concourse.bass as bass
import concourse.tile as tile
from concourse import mybir
from concourse.bass_utils import run_bass_kernel_spmd

# ---- problem constants (hardcoded per contract) ----
T = 32768
K = 16
NC = 8
TC = T // NC            # 4096 timesteps per core
G = 8                   # partition groups
SPG = 64                # sub-chunks per group -> 512 columns/core
COLS = G * SPG
L = TC // COLS          # 8 real steps per column
B = 8                   # burn-in steps
STEPS = B + L           # 16
WWIN = STEPS            # window rows per column (16)
NCHUNK = 4              # preamble pipeline chunks (4 w's each)
RS_L2 = 42              # rescale factor 2^-42 applied once at tau=B
START = 14
STOP = 15
NST = 2                 # scan streams (split over s')
SH = SPG // NST         # 32 columns per stream
FDT = mybir.dt.float32
BDT = mybir.dt.bfloat16
FWIN = (COLS - 1) * L + WWIN   # 4104 feats rows per core

_CACHE: dict = {}


def _build_kernel():
    nc = bacc.Bacc("TRN2", target_bir_lowering=False, debug=False, num_devices=NC)

    featsw = nc.dram_tensor("featsw", [FWIN, K], FDT, kind="ExternalInput").ap()
    tagsw = nc.dram_tensor("tagsw", [TC + 1], FDT, kind="ExternalInput").ap()
    transTB = nc.dram_tensor("transTB", [128, 128], FDT, kind="ExternalInput").ap()
    consts = nc.dram_tensor("consts", [128, 188], FDT, kind="ExternalInput").ap()
    out = nc.dram_tensor("out", [G, 4], FDT, kind="ExternalOutput").ap()

    with tile.TileContext(nc) as tc:
        with (
            tc.tile_pool(name="singles", bufs=1) as singles,
            tc.tile_pool(name="qps", bufs=2, space="PSUM") as qps,
            tc.tile_pool(name="dbp", bufs=1, space="PSUM") as dbp,
            tc.tile_pool(name="gps", bufs=1, space="PSUM") as gps,
            tc.tile_pool(name="sps", bufs=2, space="PSUM") as sps,
        ):
            # ---------------- small loads + constants (host-packed) ----------
            # consts cols: 0:128 ident128 | 128:144 iota16f | 144:152 gself
            #   | 152:154 initmv | 154:155 ones | 155:156 sel8(rows0..7)
            #   | 156:188.. trid rows 0:16 cols 156..188? -> trid packed at
            #   [0:16, 136:168] of a second region; see host packing below.
            csb = singles.tile([128, 188], FDT)
            nc.scalar.dma_start(out=csb, in_=consts)
            transTB_sb = singles.tile([128, 128], FDT)
            nc.sync.dma_start(out=transTB_sb, in_=transTB)
            ident128 = csb[:, 0:128]
            iota16f = csb[:, 128:144]
            gself = csb[:, 144:152]
            initmv_sb = csb[:, 152:154]
            ones16 = csb[0:K, 154:155]
            ones8 = csb[0:G, 154:155]
            sel8 = csb[0:G, 155:156]
            trid_sb = csb[0:K, 156:156 + 2 * K]
            gsel = singles.tile([128, G], BDT)
            nc.vector.tensor_copy(gsel, gself)
            ident64b = singles.tile([SPG, SPG], BDT)
            nc.vector.tensor_copy(ident64b, csb[0:SPG, 0:SPG])
            initmv_b = singles.tile([128, 2], BDT)
            nc.vector.tensor_copy(initmv_b, initmv_sb)
            # ETB = exp(transTB): block-diagonal E^T stack, bf16 for 1-pass MMs.
            # First ACT op -> exp table load overlaps the big feats DMAs.
            ETB = singles.tile([128, 128], BDT)
            nc.scalar.activation(ETB, transTB_sb, mybir.ActivationFunctionType.Exp)

            # gold-side loads on the scalar-engine DMA queue (parallel to sync)
            tsb = singles.tile([128, 33], FDT)
            nc.gpsimd.dma_start(
                out=tsb,
                in_=bass.AP(tensor=tagsw.tensor, offset=0,
                            ap=[[32, 128], [1, 33]]),
            )
            OpF = singles.tile([128, 32, 2 * K], BDT)
            OpFf = singles.tile([128, 32, K], FDT)
            nc.gpsimd.dma_start(
                out=OpFf,
                in_=bass.AP(tensor=featsw.tensor, offset=B * K,
                            ap=[[32 * K, 128], [K, 32], [1, K]]),
            )
            nc.vector.tensor_copy(OpF[:, :, K:2 * K], OpFf)

            # ---------------- gold (preamble: PE/DVE otherwise idle) ----------
            O = singles.tile([128, 32, K], BDT)
            nc.vector.tensor_tensor(
                O, tsb[:, 1:33].unsqueeze(2).broadcast_to([128, 32, K]),
                iota16f.unsqueeze(1).broadcast_to([128, 32, K]),
                mybir.AluOpType.is_equal)
            nc.vector.tensor_tensor(
                OpF[:, :, 0:K],
                tsb[:, 0:32].unsqueeze(2).broadcast_to([128, 32, K]),
                iota16f.unsqueeze(1).broadcast_to([128, 32, K]),
                mybir.AluOpType.is_equal)
            g_ps = gps.tile([K, 2 * K], FDT)
            for w in range(32):
                nc.tensor.matmul(g_ps, O[:, w, :], OpF[:, w, :],
                                 start=(w == 0), stop=(w == 31))
            gtmp = singles.tile([K, 2 * K], FDT)
            gacc = singles.tile([K, 1], FDT)
            nc.vector.tensor_tensor(gtmp, g_ps, trid_sb, mybir.AluOpType.mult)
            nc.vector.reduce_sum(gacc, gtmp, axis=mybir.AxisListType.X)
            gp_ps = sps.tile([1, 1], FDT, tag="sp")
            nc.tensor.matmul(gp_ps, ones16, gacc, start=True, stop=True)

            # ---------------- feats window: DMA -> exp -> transpose to PSUM ----
            # column c=(g*SPG+s') covers t in [base+c*L, +L); window rows
            # w in [0,16) map to featsw row c*L + w (base offset -B applied
            # on host via zero-padding).
            raww = singles.tile([SPG, G, WWIN, K], FDT)     # [64, 8, 16, 16]
            expw = singles.tile([SPG, WWIN, G, K], BDT)     # (g,i) contig per w
            dbt0 = dbp.tile([128, 8, SPG], BDT, tag="db0")
            dbt1 = dbp.tile([128, 8, SPG], BDT, tag="db1")
            dbt = [dbt0, dbt1]
            dbs = singles.tile([128, WWIN, SPG], BDT)
            CW = WWIN // NCHUNK                              # 4 w's per chunk
            GH = G // 2
            nc.sync.dma_start(
                out=raww[:, 0:GH, :, :],
                in_=bass.AP(tensor=featsw.tensor, offset=0,
                            ap=[[L * K, SPG], [SPG * L * K, GH],
                                [1, WWIN * K]]),
            )
            nc.scalar.dma_start(
                out=raww[:, GH:G, :, :],
                in_=bass.AP(tensor=featsw.tensor, offset=GH * SPG * L * K,
                            ap=[[L * K, SPG], [SPG * L * K, GH],
                                [1, WWIN * K]]),
            )
            for c in range(NCHUNK):
                nc.scalar.activation(
                    expw[:, c * CW:(c + 1) * CW, :, :].transpose([0, 2, 1, 3]),
                    raww[:, :, c * CW:(c + 1) * CW, :],
                    mybir.ActivationFunctionType.Exp)
                for w in range(c * CW, (c + 1) * CW):
                    # [64, (g,i)=128] -> [128, 64] slice of PSUM D tile
                    nc.tensor.transpose(
                        dbt[w // 8][:, w % 8, :],
                        expw[:, w, :, :],
                        ident64b)
                nc.scalar.copy(
                    dbs[:, c * CW:(c + 1) * CW, :],
                    dbt[(c * CW) // 8][:, (c * CW) % 8:(c * CW) % 8 + CW, :])

            # ---------------- scan ----------------
            Pb = singles.tile([128, SPG], BDT)
            nc.vector.memset(Pb, 1.0)
            asb = singles.tile([G, SPG], FDT)
            bsb = singles.tile([G, SPG], FDT)
            ln_a = singles.tile([G, SPG], FDT)
            ln_b = singles.tile([G, SPG], FDT)
            sa = singles.tile([G, 1], FDT)
            sb2 = singles.tile([G, 1], FDT)

            rs_const = float(2.0 ** (-RS_L2))
            for tau in range(STEPS):
                if tau == B:
                    nc.vector.tensor_scalar_mul(Pb, Pb, rs_const)
                    # core 0 only (mask/value inputs): column (g=0, s'=0)
                    nc.vector.tensor_tensor(Pb[:, 0:1], Pb[:, 0:1],
                                            initmv_b[:, 0:1],
                                            mybir.AluOpType.mult)
                    nc.vector.tensor_add(Pb[:, 0:1], Pb[:, 0:1],
                                         initmv_b[:, 1:2])
                    alpha_ps = sps.tile([G, SPG], FDT, tag="sp")
                    nc.tensor.matmul(alpha_ps, gsel, Pb, start=True, stop=True)
                    nc.vector.tensor_copy(asb, alpha_ps)
                    nc.scalar.activation(ln_a, asb,
                                         mybir.ActivationFunctionType.Ln,
                                         accum_out=sa)
                for h in range(NST):
                    Ph = Pb[:, h * SH:(h + 1) * SH]
                    q = qps.tile([128, SH], FDT, tag="q")
                    nc.tensor.matmul(q, ETB, Ph, start=True, stop=True)
                    dsl = dbs[:, tau, h * SH:(h + 1) * SH]
                    nc.vector.tensor_tensor(Ph, q, dsl, mybir.AluOpType.mult)

            beta_ps = sps.tile([G, SPG], FDT, tag="sp")
            nc.tensor.matmul(beta_ps, gsel, Pb, start=True, stop=True)
            nc.vector.tensor_copy(bsb, beta_ps)

            # ---------------- epilogue ----------------
            nc.scalar.activation(ln_b, bsb, mybir.ActivationFunctionType.Ln,
                                 accum_out=sb2)
            d8 = singles.tile([G, 1], FDT)
            nc.vector.tensor_sub(d8, sb2, sa)

            # u . v_end: u = ETB[:, 127] (block g=7, row STOP); beta_last via sel8
            ud_ps = sps.tile([1, 1], FDT, tag="sp")
            nc.tensor.matmul(ud_ps, ETB[:, 127:128], Pb[:, SPG - 1:SPG],
                             start=True, stop=True)
            bl_ps = sps.tile([1, 1], FDT, tag="sp")
            nc.tensor.matmul(bl_ps, sel8, bsb[:, SPG - 1:SPG],
                             start=True, stop=True)

            osb = singles.tile([G, 4], FDT)
            nc.vector.memset(osb, 0.0)
            nc.vector.tensor_copy(osb[:, 0:1], d8)
            nc.vector.tensor_copy(osb[0:1, 1:2], ud_ps)
            nc.vector.tensor_copy(osb[0:1, 2:3], bl_ps)
            nc.vector.tensor_copy(osb[0:1, 3:4], gp_ps)
            nc.sync.dma_start(out=out, in_=osb)

    nc.compile()
    return nc


def _get_nc():
    if "nc" not in _CACHE:
        _CACHE["nc"] = _build_kernel()
    return _CACHE["nc"]


def _make_in_maps(feats, tags, transitions):
    feats = np.ascontiguousarray(feats, dtype=np.float32)
    tags_i = np.asarray(tags).astype(np.int64)
    trans = np.ascontiguousarray(transitions, dtype=np.float32)

    featsP = np.vstack([np.zeros((B, K), np.float32), feats])
    tagsX = np.concatenate([[START], tags_i]).astype(np.float32)
    # block-diagonal transposed-transitions pattern; exp of -1e4 -> 0 off-blocks
    TB = np.full((128, 128), -10000.0, np.float32)
    for g in range(G):
        TB[g * K:(g + 1) * K, g * K:(g + 1) * K] = trans.T

    base_consts = np.zeros((128, 188), np.float32)
    base_consts[:, 0:128] = np.eye(128, dtype=np.float32)
    base_consts[:, 128:144] = np.arange(K, dtype=np.float32)[None, :]
    base_consts[:, 144:152] = (np.arange(128)[:, None] // K ==
                               np.arange(G)[None, :]).astype(np.float32)
    base_consts[:, 152] = 1.0              # initmv mask (non-core-0 default)
    base_consts[:, 154] = 1.0              # ones
    base_consts[7, 155] = 1.0              # sel8
    base_consts[0:K, 156:172] = trans
    base_consts[0:K, 172:188] = np.eye(K, dtype=np.float32)

    in_maps = []
    for c in range(NC):
        base = c * TC
        cc = base_consts.copy()
        if c == 0:
            cc[0:K, 152] = 0.0
            cc[START, 153] = 1.0       # column (g=0, s'=0) -> e_START
        in_maps.append({
            "featsw": np.ascontiguousarray(featsP[base: base + FWIN]),
            "tagsw": np.ascontiguousarray(tagsX[base: base + TC + 1]),
            "transTB": TB,
            "consts": cc,
        })
    return in_maps, tags_i, trans


def _combine(outs, tags_i, trans):
    fwd = sum(float(o["out"][:, 0].sum()) for o in outs)
    logZ = fwd + math.log(float(outs[-1]["out"][0, 1])) \
        - math.log(float(outs[-1]["out"][0, 2]))
    gold = sum(float(o["out"][0, 3]) for o in outs)
    gold += float(trans[STOP, tags_i[-1]])
    return np.float32((logZ - gold) / T)


def kernel(feats, tags, transitions):
    nc = _get_nc()
    in_maps, tags_i, trans = _make_in_maps(feats, tags, transitions)
    res = run_bass_kernel_spmd(nc, in_maps, core_ids=list(range(NC)))
    return _combine(res.results, tags_i, trans)


if __name__ == "__main__":
    d = np.load("/root/problem/inputs_only.npz")
    loss = kernel(d["feats"], d["tags"], d["transitions"])
    print("loss:", loss)
